# revision 50
# baseline (speedup 1.0000x reference)
"""CRF partition function (neg log partition) on 8 Trainium2 NeuronCores.

Algorithm: rank-1 chunked scan. In prob space p_t = p_{t-1} @ (E D_t) with
E = exp(log_transition) row-stochastic and D_t = diag(exp(obs_t - beta)).
Products of positive matrices contract to rank-1 fast (E is dense softmax),
so the T=4096 serial scan splits into C = T/L independent chunks of L=8
steps run in parallel as columns of the [S, N=4096] state. Each chunk
starts from the all-ones probe with NO warmup (W=0): the per-chunk scale
ratios are stitched on the host in fp64 from device column sums, and the
probe-direction error cancels in the telescoped ly/lw ratios (validated
~8e-4 rel err vs f64 reference, tolerance 2e-2; W>0 supported but
measurably unnecessary).

Device layout (per core, 8 batches): chains n = c*BPC + b; G = L slices;
per slice X <- (E^T X) * e_slice. Columns are partitioned into subgroups,
each an independent software pipeline (own state tile, own PSUM pool):
  kind "a": ACT copies PSUM->SBUF bf16, DVE multiplies in 2x mode (es bf16)
  kind "d": DVE tensor_mul directly from PSUM (es fp8)
This splits the PSUM-evacuate+multiply work across ACT+DVE (DVE alone is
the bottleneck otherwise; GPSIMD cannot read PSUM, and ACT-evac+Pool-mul
adds too much chain latency). fp8 e-slices halve DMA for the "d" groups,
whose multiply gains nothing from bf16.

L=8 with SINGLE-buffered PSUM pools ([a1024 x3, d512, d512] = exactly 8
banks) beats the L=16 double-buffered layouts: wide slices amortize the
per-instruction fixed costs and the mm->ACT->DVE chain latency (~2.4us)
fits inside the ~3.1us slice period (ACT-saturated: back-to-back 1.04us
evacs), so psb=1's serialization never binds.

e-slices are host-precomputed exp(obs - beta) (beta 0.5 for bf16 groups;
0.0 with clipping to the e4m3 range for fp8 groups), pre-transposed into
[S, G*width] layout and fetched one slice per DMA (the pipeline starts on
the first slice's arrival). Chunk 0 is exact: its state is initialized
with p_0 = exp(obs[:,0]-beta) via a Pool-engine copy. The final chunk
consumes one pad column e=1 (exact: E is row-stochastic). The host stitch
removes the beta bias per chunk; with W=0 the warmup sum is exactly S, so
only the final state (yout) leaves the device.
"""

import numpy as np
import ml_dtypes

import concourse.bacc as bacc
import concourse.mybir as mybir
from concourse.tile import TileContext
from concourse.bass_utils import run_bass_kernel_spmd

bf16 = ml_dtypes.bfloat16
fp8 = ml_dtypes.float8_e4m3

B, T, S = 64, 4096, 128
NCORES = 8
BPC = B // NCORES     # 8 batches per core

BETA16 = 0.5          # bias for bf16 groups
BETA8 = 0.0           # bias for fp8 groups (centers e4m3 range)
CLIP_LO = 2.0 ** -8
CLIP_HI = 224.0

# ---- configuration ----
WOUT_BIG = True
PE_PREWARM = 0
YOUT_BIG = False
ES16_FIRST = False
E0_DEFER = False
SHARED_PS = False
SHARED_PS_BUFS = 2
MEMSET_SPLIT = False
SPLIT_HEAD = 1
BATCH_HEAD = (1, 1, 1, 1, 1, 1, 1, 1)
L, W, KD = 8, 0, 2
LAYOUT = [("a", 1024), ("a", 1024), ("a", 1024), ("d", 512), ("d", 512)]
PSBUFS = [1, 1, 1, 1, 1]

C = T // L
G = L + W
N = C * BPC


def configure(l, w, layout=None, kd=None, psbufs=None):
    global L, W, C, G, N, LAYOUT, KD, PSBUFS
    L, W = l, w
    C = T // L
    G = L + W
    N = C * BPC
    if layout is not None:
        LAYOUT = layout
    if kd is not None:
        KD = kd
    if psbufs is not None:
        PSBUFS = psbufs
    assert sum(e[1] for e in LAYOUT) == N, (sum(e[1] for e in LAYOUT), N)
    assert all(e[1] % BPC == 0 for e in LAYOUT)
    assert all(len(e) == 2 or e[2] % BPC == 0 for e in LAYOUT)


def _psbufs():
    if PSBUFS is not None and len(PSBUFS) == len(LAYOUT):
        return PSBUFS
    return [2] * len(LAYOUT)


def _batches():
    """Variable-size es DMA batches: tiny first so compute starts early."""
    sizes = []
    for s in BATCH_HEAD:
        if sum(sizes) + s <= G:
            sizes.append(s)
    while sum(sizes) < G:
        sizes.append(min(KD, G - sum(sizes)))
    starts = np.cumsum([0] + sizes[:-1]).tolist()
    return list(zip(starts, sizes))


def _group_geometry():
    """kinds: "d" DVE-direct (es fp8), "a" ACT-evac+DVE-2x (es bf16),
    "m" mixed tile: cols [0:x] d-style, [x:w] a-style (shares one PSUM tile,
    balancing DVE vs ACT at finer granularity than the bank-rounded tiles)."""
    geo = []
    col = o16 = o8 = 0
    for ent in LAYOUT:
        kind, w = ent[0], ent[1]
        if kind == "a":
            geo.append({"kind": kind, "w": w, "col": col, "eoff": o16})
            o16 += w
        elif kind == "d":
            geo.append({"kind": kind, "w": w, "col": col, "eoff": o8})
            o8 += w
        else:
            x = ent[2]
            geo.append({"kind": "m", "w": w, "col": col, "x": x,
                        "eoff": o8, "eoff16": o16})
            o8 += x
            o16 += w - x
        col += w
    return geo, o16, o8


def _chunk_slot(geo, c):
    """Map chunk c -> (dtype_tag, col offset within that dtype's es tensor)."""
    col = c * BPC
    for sub in geo:
        if sub["col"] <= col < sub["col"] + sub["w"]:
            off = col - sub["col"]
            if sub["kind"] == "a":
                return "16", sub["eoff"] + off
            if sub["kind"] == "d":
                return "8", sub["eoff"] + off
            if off < sub["x"]:
                return "8", sub["eoff"] + off
            return "16", sub["eoff16"] + off - sub["x"]
    raise AssertionError(c)


def _build_device_program(esbufs=4, xbufs=4, evbufs=4):
    geo, A16, A8 = _group_geometry()
    psb = _psbufs()
    batches = _batches()
    nc = bacc.Bacc("TRN2", target_bir_lowering=False)
    # es layout: [S, G*width]; a batch of s slices starting at g0 is the
    # column range [g0*width, (g0+s)*width) -- contiguous per partition row
    es16_d = nc.dram_tensor("es16", [S, G * A16], mybir.dt.bfloat16,
                            kind="ExternalInput") if A16 else None
    es8_d = nc.dram_tensor("es8", [S, G * A8], mybir.dt.float8e4,
                           kind="ExternalInput") if A8 else None
    cst_d = nc.dram_tensor("cst", [S, S + BPC], mybir.dt.bfloat16,
                           kind="ExternalInput")
    wout_d = nc.dram_tensor("wout", [S, N], mybir.dt.bfloat16, kind="ExternalOutput")
    yout_d = nc.dram_tensor("yout", [S, N], mybir.dt.bfloat16, kind="ExternalOutput")

    from contextlib import ExitStack
    with TileContext(nc) as tc, ExitStack() as stk:
        cpool = stk.enter_context(tc.tile_pool(name="const", bufs=1))
        e16pool = stk.enter_context(tc.tile_pool(name="es16", bufs=esbufs))
        e8pool = stk.enter_context(tc.tile_pool(name="es8", bufs=esbufs))
        xpool = stk.enter_context(tc.tile_pool(name="state", bufs=xbufs))
        evpool = stk.enter_context(tc.tile_pool(name="ev", bufs=evbufs))
        if SHARED_PS == "a":
            pshared = stk.enter_context(
                tc.tile_pool(name="psa", bufs=SHARED_PS_BUFS, space="PSUM"))
            ppools = [
                (None if sub["kind"] == "a" else stk.enter_context(
                    tc.tile_pool(name=f"ps{gi}", bufs=psb[gi], space="PSUM")))
                for gi, sub in enumerate(geo)
            ]
        elif SHARED_PS:
            pshared = stk.enter_context(
                tc.tile_pool(name="ps", bufs=SHARED_PS_BUFS, space="PSUM"))
            ppools = None
        else:
            ppools = [
                stk.enter_context(
                    tc.tile_pool(name=f"ps{gi}", bufs=pb, space="PSUM"))
                for gi, pb in enumerate(psb)
            ]

        cst_sb = cpool.tile([S, S + BPC], mybir.dt.bfloat16)
        nc.sync.dma_start(out=cst_sb[:], in_=cst_d[:])
        E_sb = cst_sb[:, 0:S]
        e0_sb = cst_sb[:, S:S + BPC]

        if PE_PREWARM:
            # tiny dependency-free matmuls issued first: PE's clock ramps with
            # continuous busy time, so the first real matmuls run full speed
            junk = cpool.tile([S, 16], mybir.dt.bfloat16)
            nc.vector.memset(junk[:], 1.0)
            jps = ppools[0].tile([16, 16], mybir.dt.float32, tag="ps0", name="jps")
            for _ in range(PE_PREWARM):
                nc.tensor.matmul(out=jps[:], lhsT=junk[:], rhs=junk[:],
                                 start=True, stop=True)

        # independent per-subgroup state tiles; init on Pool (DVE stays free).
        # "m" subgroups keep two tiles (d-part, a-part) so each part's writer
        # feeds its own readers -- one shared tile with two writers serializes
        # the whole pipeline under tile-granular dependency tracking.
        def new_state(gi, sub, g=None):
            sfx = "" if g is None else f"_{g}"
            if sub["kind"] == "m":
                xd = xpool.tile([S, sub["x"]], mybir.dt.bfloat16,
                                tag=f"xd{gi}", name=f"xd{gi}{sfx}")
                xa = xpool.tile([S, sub["w"] - sub["x"]], mybir.dt.bfloat16,
                                tag=f"xa{gi}", name=f"xa{gi}{sfx}")
                return (xd[:], xa[:])
            xt = xpool.tile([S, sub["w"]], mybir.dt.bfloat16,
                            tag=f"x{gi}", name=f"x{gi}{sfx}")
            return xt[:]

        def state_seg(xs, sub, k0, k1):
            """rhs AP for matmul segment [k0:k1) of this subgroup's state."""
            if sub["kind"] != "m":
                return xs[:, k0:k1]
            x = sub["x"]
            if k1 <= x:
                return xs[0][:, k0:k1]
            assert k0 >= x, (k0, k1, x)
            return xs[1][:, k0 - x:k1 - x]

        def seg_bounds(sub):
            w = sub["w"]
            cuts = {0, w}
            cuts.update(range(512, w, 512))
            if sub["kind"] == "m":
                cuts.add(sub["x"])
            cs = sorted(cuts)
            return list(zip(cs[:-1], cs[1:]))

        X = []
        for gi, sub in enumerate(geo):
            xs = new_state(gi, sub)
            for part in (xs if isinstance(xs, tuple) else (xs,)):
                # d-inits on DVE (their pipelines start first; Pool's serial
                # memset queue would gate the first matmuls), a-inits on Pool
                if sub["kind"] == "d" and MEMSET_SPLIT:
                    nc.vector.memset(part, 1.0)
                else:
                    nc.gpsimd.memset(part, 1.0)
            X.append(xs)
        if W == 0:
            # no warmup: exact chunk-0 init goes straight into the initial state
            x0 = X[0][0] if isinstance(X[0], tuple) else X[0]
            nc.gpsimd.tensor_copy(out=x0[:, 0:BPC], in_=e0_sb)

        e16_t = e8_t = None
        bi = -1
        for g in range(G):
            hit = [b for b, (g0, s) in enumerate(batches) if g0 == g]
            if hit:
                bi = hit[0]
                g0, bs = batches[bi]
                if es16_d is not None and ES16_FIRST:
                    e16_t = e16pool.tile([S, KD * A16], mybir.dt.bfloat16)
                    nc.sync.dma_start(out=e16_t[:, 0:bs * A16],
                                      in_=es16_d[:, g0 * A16:(g0 + bs) * A16])
                if es8_d is not None:
                    e8_t = e8pool.tile([S, KD * A8], mybir.dt.float8e4)
                    nc.sync.dma_start(out=e8_t[:, 0:bs * A8],
                                      in_=es8_d[:, g0 * A8:(g0 + bs) * A8])
                if es16_d is not None and not ES16_FIRST:
                    e16_t = e16pool.tile([S, KD * A16], mybir.dt.bfloat16)
                    if bi < SPLIT_HEAD and bs == 1:
                        # per-group transfers for the leading slices: each
                        # pipeline starts on its own (much smaller) arrival
                        for sub in geo:
                            if sub["kind"] != "a":
                                continue
                            eo, wsub = sub["eoff"], sub["w"]
                            nc.sync.dma_start(
                                out=e16_t[:, eo:eo + wsub],
                                in_=es16_d[:, g0 * A16 + eo:g0 * A16 + eo + wsub])
                    else:
                        nc.sync.dma_start(out=e16_t[:, 0:bs * A16],
                                          in_=es16_d[:, g0 * A16:(g0 + bs) * A16])
            j = g - batches[bi][0]

            final = (W > 0 and g == W - 1 and WOUT_BIG) or (g == G - 1 and YOUT_BIG)
            if final:
                # single wide tile so wout/yout go out as one DMA each
                Xbig = xpool.tile([S, N], mybir.dt.bfloat16, tag=f"xfin{g}")
            if SHARED_PS == "a":
                a_tot = sum(s["w"] for s in geo if s["kind"] == "a")
                psbig = pshared.tile([S, a_tot], mybir.dt.float32, tag="psa")
            elif SHARED_PS:
                psbig = pshared.tile([S, N], mybir.dt.float32, tag="ps")
            aoff = 0
            for gi, sub in enumerate(geo):
                w, col, eo, kind = sub["w"], sub["col"], sub["eoff"], sub["kind"]
                if SHARED_PS == "a" and kind == "a":
                    ps = psbig[:, aoff:aoff + w]
                    aoff += w
                elif SHARED_PS is True:
                    ps = psbig[:, col:col + w]
                else:
                    pst = ppools[gi].tile([S, w], mybir.dt.float32, tag=f"ps{gi}")
                    ps = pst[:]
                for k0, k1 in seg_bounds(sub):
                    nc.tensor.matmul(out=ps[:, k0:k1], lhsT=E_sb,
                                     rhs=state_seg(X[gi], sub, k0, k1),
                                     start=True, stop=True)
                if final and kind != "m":
                    Xn = Xbig[:, col:col + w]
                else:
                    Xn = new_state(gi, sub, g)
                if kind == "d":
                    es = e8_t[:, j * A8 + eo: j * A8 + eo + w]
                    nc.vector.tensor_mul(out=Xn, in0=ps, in1=es)
                elif kind == "a":
                    ev = evpool.tile([S, w], mybir.dt.bfloat16, tag=f"ev{gi}")
                    nc.scalar.copy(out=ev[:], in_=ps)
                    es = e16_t[:, j * A16 + eo: j * A16 + eo + w]
                    nc.vector.tensor_mul(out=Xn, in0=ev[:], in1=es)
                else:  # "m": d-style [0:x] and a-style [x:w] parts
                    x, eo16 = sub["x"], sub["eoff16"]
                    es = e8_t[:, j * A8 + eo: j * A8 + eo + x]
                    nc.vector.tensor_mul(out=Xn[0], in0=ps[:, 0:x], in1=es)
                    ev = evpool.tile([S, w - x], mybir.dt.bfloat16, tag=f"ev{gi}")
                    nc.scalar.copy(out=ev[:], in_=ps[:, x:w])
                    es = e16_t[:, j * A16 + eo16: j * A16 + eo16 + (w - x)]
                    nc.vector.tensor_mul(out=Xn[1], in0=ev[:], in1=es)
                X[gi] = Xn
            if W > 0 and g == W - 1:
                # exact chunk-0 init via Pool copy (keeps SP/HWDGE free); the
                # wout snapshot skips cols 0:BPC (unused in the stitch) so the
                # two touch disjoint regions and don't serialize
                nc.gpsimd.tensor_copy(out=X[0][:, 0:BPC], in_=e0_sb)
                wsnap = list(X)
            if W > 0 and g == W + 1:
                # wout issued 2 slices late so its sem wait never starves the
                # SP queue of upcoming es batches
                for gi, sub in enumerate(geo):
                    lo = BPC if gi == 0 else 0
                    nc.sync.dma_start(
                        out=wout_d[:, sub["col"] + lo:sub["col"] + sub["w"]],
                        in_=wsnap[gi][:, lo:sub["w"]])
        if YOUT_BIG and not any(s["kind"] == "m" for s in geo):
            nc.sync.dma_start(out=yout_d[:], in_=Xbig[:])
        else:
            for gi, sub in enumerate(geo):
                c0 = sub["col"]
                if sub["kind"] == "m":
                    x = sub["x"]
                    nc.sync.dma_start(out=yout_d[:, c0:c0 + x], in_=X[gi][0])
                    nc.sync.dma_start(out=yout_d[:, c0 + x:c0 + sub["w"]],
                                      in_=X[gi][1])
                else:
                    nc.sync.dma_start(
                        out=yout_d[:, c0:c0 + sub["w"]], in_=X[gi])
    nc.compile()
    return nc


LAST_RESULTS = None
LAST_NC = None


def _chunk_meta():
    geo, _, _ = _group_geometry()
    beta = np.empty(C)
    for c in range(C):
        tag, _ = _chunk_slot(geo, c)
        beta[c] = BETA16 if tag == "16" else BETA8
    return beta


def prep_in_maps(log_observation: np.ndarray, log_transition: np.ndarray):
    geo, A16, A8 = _group_geometry()
    E = np.exp(log_transition.astype(np.float64)).astype(np.float32)
    ew_bf = E.astype(bf16)

    e8_full = np.exp(log_observation.astype(np.float32) - BETA8)  # [B,T,S] f32
    sc = np.float32(np.exp(BETA8 - BETA16))

    in_maps = []
    for k in range(NCORES):
        blk8 = e8_full[k * BPC:(k + 1) * BPC]          # [BPC, T, S] f32
        if W > 0:
            pad8 = np.ones((S, BPC, T + W), dtype=np.float32)
            pad8[:, :, W - 1:W - 1 + T] = blk8.transpose(2, 0, 1)
        else:
            # t = L*c + g + 1: slice stream starts at obs[:,1]; obs[:,0] is
            # consumed by the exact e0 init
            pad8 = np.ones((S, BPC, T), dtype=np.float32)
            pad8[:, :, 0:T - 1] = blk8[:, 1:, :].transpose(2, 0, 1)
        st_s, st_b, st_t = pad8.strides
        view = np.lib.stride_tricks.as_strided(
            pad8, shape=(G, S, C, BPC), strides=(st_t, st_s, L * st_t, st_b)
        )  # [g, s, c, b] fp32

        es16 = np.ones((G, S, A16), dtype=bf16) if A16 else None
        es8 = np.ones((G, S, A8), dtype=fp8) if A8 else None
        for sub in geo:
            c0 = sub["col"] // BPC
            parts = [(sub["kind"], 0, sub["w"], sub["eoff"])]
            if sub["kind"] == "m":
                x = sub["x"]
                parts = [("d", 0, x, sub["eoff"]),
                         ("a", x, sub["w"] - x, sub["eoff16"])]
            for knd, off, pw, eo in parts:
                v = view[:, :, c0 + off // BPC:c0 + (off + pw) // BPC, :]
                v = v.reshape(G, S, pw)
                if knd == "a":
                    es16[:, :, eo:eo + pw] = (v * sc).astype(bf16)
                else:
                    es8[:, :, eo:eo + pw] = np.clip(
                        v, CLIP_LO, CLIP_HI).astype(fp8)
        # the final chunk's g=G-1 step is a pad: force its factor to exactly 1
        tag, lo = _chunk_slot(geo, C - 1)
        if tag == "16":
            es16[G - 1, :, lo:lo + BPC] = 1.0
        else:
            es8[G - 1, :, lo:lo + BPC] = 1.0
        m = {}
        if A16:
            m["es16"] = np.ascontiguousarray(
                es16.transpose(1, 0, 2)).reshape(S, G * A16)
        if A8:
            m["es8"] = np.ascontiguousarray(
                es8.transpose(1, 0, 2)).reshape(S, G * A8)
        beta0 = BETA16 if _chunk_slot(geo, 0)[0] == "16" else BETA8
        e0 = np.exp(log_observation[k * BPC:(k + 1) * BPC, 0, :].astype(np.float32).T
                    - beta0)
        m["cst"] = np.concatenate([ew_bf, e0.astype(bf16)], axis=1)
        in_maps.append(m)
    return in_maps


def stitch_outputs(results) -> np.ndarray:
    betc = _chunk_meta()                    # [C]
    cnt = np.full(C, G, dtype=np.float64)
    cnt[0] = L + 1
    cnt[C - 1] = G - 1
    out = np.empty(B, dtype=np.float64)
    for k in range(NCORES):
        y = results[k]["yout"].astype(np.float64).reshape(S, C, BPC)
        Sy = y.sum(axis=0)            # [C, BPC]
        if W == 0:
            Sw = np.full((C, BPC), float(S))
        else:
            w = results[k]["wout"].astype(np.float64).reshape(S, C, BPC)
            Sw = np.maximum(w.sum(axis=0), 1e-300)  # chunk-0 cols unused
        ly = np.log(Sy) + (betc * cnt)[:, None]
        lw = np.log(Sw) + (betc * W)[:, None]
        logZ = ly[C - 1] + np.sum(ly[: C - 1] - lw[1:], axis=0)
        out[k * BPC:(k + 1) * BPC] = -logZ
    return out


def kernel(log_observation: np.ndarray, log_transition: np.ndarray) -> np.ndarray:
    assert log_observation.shape == (B, T, S)
    assert log_transition.shape == (S, S)

    in_maps = prep_in_maps(log_observation, log_transition)
    nc = _build_device_program()
    res = run_bass_kernel_spmd(nc, in_maps, core_ids=list(range(NCORES)))
    global LAST_RESULTS, LAST_NC
    LAST_RESULTS = res
    LAST_NC = nc
    return stitch_outputs(res.results).astype(np.float32)


if __name__ == "__main__":
    rng = np.random.default_rng(0)
    obs = rng.standard_normal((B, T, S), dtype=np.float32)
    lt = rng.standard_normal((S, S), dtype=np.float32)
    lt = lt - np.log(np.exp(lt).sum(axis=1, keepdims=True))
    print(kernel(obs, lt)[:4])


# revision 53
# speedup vs baseline: 1.0539x; 1.0539x over previous
"""CRF partition function (neg log partition) on 8 Trainium2 NeuronCores.

Algorithm: rank-1 chunked scan. In prob space p_t = p_{t-1} @ (E D_t) with
E = exp(log_transition) row-stochastic and D_t = diag(exp(obs_t - beta)).
Products of positive matrices contract to rank-1 fast (E is dense softmax),
so the T=4096 serial scan splits into C = T/L independent chunks of L=8
steps run in parallel as columns of the [S, N=4096] state. Each chunk
starts from the all-ones probe with NO warmup (W=0): the per-chunk scale
ratios are stitched on the host in fp64 from device column sums, and the
probe-direction error cancels in the telescoped ly/lw ratios (validated
~8e-4 rel err vs f64 reference, tolerance 2e-2; W>0 supported but
measurably unnecessary).

Device layout (per core, 8 batches): chains n = c*BPC + b; G = L slices;
per slice X <- (E^T X) * e_slice. Columns are partitioned into subgroups,
each an independent software pipeline (own state tile, own PSUM pool):
  kind "a": ACT copies PSUM->SBUF bf16, DVE multiplies in 2x mode (es bf16)
  kind "d": DVE tensor_mul directly from PSUM (es fp8)
This splits the PSUM-evacuate+multiply work across ACT+DVE (DVE alone is
the bottleneck otherwise; GPSIMD cannot read PSUM, and ACT-evac+Pool-mul
adds too much chain latency). fp8 e-slices halve DMA for the "d" groups,
whose multiply gains nothing from bf16.

L=8 with SINGLE-buffered PSUM pools ([a1024 x3, d512, d512] = exactly 8
banks) beats the L=16 double-buffered layouts: wide slices amortize the
per-instruction fixed costs and the mm->ACT->DVE chain latency (~2.4us)
fits inside the ~3.1us slice period (ACT-saturated: back-to-back 1.04us
evacs), so psb=1's serialization never binds.

e-slices are host-precomputed exp(obs - beta) (beta 0.5 for bf16 groups;
0.0 with clipping to the e4m3 range for fp8 groups), pre-transposed into
[S, G*width] layout and fetched one slice per DMA (the pipeline starts on
the first slice's arrival). Chunk 0 is exact: its state is initialized
with p_0 = exp(obs[:,0]-beta) via a Pool-engine copy. The final chunk
consumes one pad column e=1 (exact: E is row-stochastic). The host stitch
removes the beta bias per chunk; with W=0 the warmup sum is exactly S, so
only the final state (yout) leaves the device.
"""

import numpy as np
import ml_dtypes

import concourse.bacc as bacc
import concourse.mybir as mybir
from concourse.tile import TileContext
from concourse.bass_utils import run_bass_kernel_spmd

bf16 = ml_dtypes.bfloat16
fp8 = ml_dtypes.float8_e4m3

B, T, S = 64, 4096, 128
NCORES = 8
BPC = B // NCORES     # 8 batches per core

BETA16 = 0.5          # bias for bf16 groups
BETA8 = 0.0           # bias for fp8 groups (centers e4m3 range)
CLIP_LO = 2.0 ** -8
CLIP_HI = 224.0

# ---- configuration ----
WOUT_BIG = True
PE_PREWARM = 0
YOUT_BIG = False
ES16_FIRST = False
E0_DEFER = False
SHARED_PS = False
SHARED_PS_BUFS = 2
MEMSET_SPLIT = False
SPLIT_HEAD = 1
BATCH_HEAD = (1, 1, 1, 1, 1, 1, 1, 1)
L, W, KD = 8, 0, 2
LAYOUT = [("q", 1024, 656), ("q", 1024, 656), ("q", 512, 0),
          ("d", 512), ("d", 512), ("d", 512)]
PSBUFS = [1, 1, 1, 1, 1, 1]

C = T // L
G = L + W
N = C * BPC


def configure(l, w, layout=None, kd=None, psbufs=None):
    global L, W, C, G, N, LAYOUT, KD, PSBUFS
    L, W = l, w
    C = T // L
    G = L + W
    N = C * BPC
    if layout is not None:
        LAYOUT = layout
    if kd is not None:
        KD = kd
    if psbufs is not None:
        PSBUFS = psbufs
    assert sum(e[1] for e in LAYOUT) == N, (sum(e[1] for e in LAYOUT), N)
    assert all(e[1] % BPC == 0 for e in LAYOUT)
    assert all(len(e) == 2 or e[2] % BPC == 0 for e in LAYOUT)


def _psbufs():
    if PSBUFS is not None and len(PSBUFS) == len(LAYOUT):
        return PSBUFS
    return [2] * len(LAYOUT)


def _batches():
    """Variable-size es DMA batches: tiny first so compute starts early."""
    sizes = []
    for s in BATCH_HEAD:
        if sum(sizes) + s <= G:
            sizes.append(s)
    while sum(sizes) < G:
        sizes.append(min(KD, G - sum(sizes)))
    starts = np.cumsum([0] + sizes[:-1]).tolist()
    return list(zip(starts, sizes))


def _group_geometry():
    """kinds: "d" DVE-direct (es fp8), "a" ACT-evac+DVE-2x (es bf16),
    "m" mixed tile: cols [0:x] d-style, [x:w] a-style (shares one PSUM tile,
    balancing DVE vs ACT at finer granularity than the bank-rounded tiles)."""
    geo = []
    col = o16 = o8 = 0
    for ent in LAYOUT:
        kind, w = ent[0], ent[1]
        if kind == "a":
            geo.append({"kind": kind, "w": w, "col": col, "eoff": o16})
            o16 += w
        elif kind == "d":
            geo.append({"kind": kind, "w": w, "col": col, "eoff": o8})
            o8 += w
        elif kind == "m":
            x = ent[2]
            geo.append({"kind": "m", "w": w, "col": col, "x": x,
                        "eoff": o8, "eoff16": o16})
            o8 += x
            o16 += w - x
        else:  # "q": ACT evacuates all w; DVE 2x-muls [0:aw], Pool muls [aw:w]
            aw = ent[2]
            geo.append({"kind": "q", "w": w, "col": col, "x": aw,
                        "eoff": o16, "eoff8": o8})
            o16 += aw
            o8 += w - aw
        col += w
    return geo, o16, o8


def _chunk_slot(geo, c):
    """Map chunk c -> (dtype_tag, col offset within that dtype's es tensor)."""
    col = c * BPC
    for sub in geo:
        if sub["col"] <= col < sub["col"] + sub["w"]:
            off = col - sub["col"]
            if sub["kind"] == "a":
                return "16", sub["eoff"] + off
            if sub["kind"] == "d":
                return "8", sub["eoff"] + off
            if sub["kind"] == "m":
                if off < sub["x"]:
                    return "8", sub["eoff"] + off
                return "16", sub["eoff16"] + off - sub["x"]
            if off < sub["x"]:  # "q" a-part
                return "16", sub["eoff"] + off
            return "8", sub["eoff8"] + off - sub["x"]
    raise AssertionError(c)


def _build_device_program(esbufs=4, xbufs=4, evbufs=4):
    geo, A16, A8 = _group_geometry()
    psb = _psbufs()
    batches = _batches()
    nc = bacc.Bacc("TRN2", target_bir_lowering=False)
    # es layout: [S, G*width]; a batch of s slices starting at g0 is the
    # column range [g0*width, (g0+s)*width) -- contiguous per partition row
    es16_d = nc.dram_tensor("es16", [S, G * A16], mybir.dt.bfloat16,
                            kind="ExternalInput") if A16 else None
    es8_d = nc.dram_tensor("es8", [S, G * A8], mybir.dt.float8e4,
                           kind="ExternalInput") if A8 else None
    cst_d = nc.dram_tensor("cst", [S, S + BPC], mybir.dt.bfloat16,
                           kind="ExternalInput")
    wout_d = nc.dram_tensor("wout", [S, N], mybir.dt.bfloat16, kind="ExternalOutput")
    yout_d = nc.dram_tensor("yout", [S, N], mybir.dt.bfloat16, kind="ExternalOutput")

    from contextlib import ExitStack
    with TileContext(nc) as tc, ExitStack() as stk:
        cpool = stk.enter_context(tc.tile_pool(name="const", bufs=1))
        e16pool = stk.enter_context(tc.tile_pool(name="es16", bufs=esbufs))
        e8pool = stk.enter_context(tc.tile_pool(name="es8", bufs=esbufs))
        xpool = stk.enter_context(tc.tile_pool(name="state", bufs=xbufs))
        evpool = stk.enter_context(tc.tile_pool(name="ev", bufs=evbufs))
        if SHARED_PS == "a":
            pshared = stk.enter_context(
                tc.tile_pool(name="psa", bufs=SHARED_PS_BUFS, space="PSUM"))
            ppools = [
                (None if sub["kind"] == "a" else stk.enter_context(
                    tc.tile_pool(name=f"ps{gi}", bufs=psb[gi], space="PSUM")))
                for gi, sub in enumerate(geo)
            ]
        elif SHARED_PS:
            pshared = stk.enter_context(
                tc.tile_pool(name="ps", bufs=SHARED_PS_BUFS, space="PSUM"))
            ppools = None
        else:
            ppools = [
                stk.enter_context(
                    tc.tile_pool(name=f"ps{gi}", bufs=pb, space="PSUM"))
                for gi, pb in enumerate(psb)
            ]

        cst_sb = cpool.tile([S, S + BPC], mybir.dt.bfloat16)
        nc.sync.dma_start(out=cst_sb[:], in_=cst_d[:])
        E_sb = cst_sb[:, 0:S]
        e0_sb = cst_sb[:, S:S + BPC]

        if PE_PREWARM:
            # tiny dependency-free matmuls issued first: PE's clock ramps with
            # continuous busy time, so the first real matmuls run full speed
            junk = cpool.tile([S, 16], mybir.dt.bfloat16)
            nc.vector.memset(junk[:], 1.0)
            jps = ppools[0].tile([16, 16], mybir.dt.float32, tag="ps0", name="jps")
            for _ in range(PE_PREWARM):
                nc.tensor.matmul(out=jps[:], lhsT=junk[:], rhs=junk[:],
                                 start=True, stop=True)

        # independent per-subgroup state tiles; init on Pool (DVE stays free).
        # "m" subgroups keep two tiles (d-part, a-part) so each part's writer
        # feeds its own readers -- one shared tile with two writers serializes
        # the whole pipeline under tile-granular dependency tracking.
        def new_state(gi, sub, g=None):
            sfx = "" if g is None else f"_{g}"
            if sub["kind"] in ("m", "q") and 0 < sub["x"] < sub["w"]:
                xd = xpool.tile([S, sub["x"]], mybir.dt.bfloat16,
                                tag=f"xd{gi}", name=f"xd{gi}{sfx}")
                xa = xpool.tile([S, sub["w"] - sub["x"]], mybir.dt.bfloat16,
                                tag=f"xa{gi}", name=f"xa{gi}{sfx}")
                return (xd[:], xa[:])
            xt = xpool.tile([S, sub["w"]], mybir.dt.bfloat16,
                            tag=f"x{gi}", name=f"x{gi}{sfx}")
            return xt[:]

        def state_seg(xs, sub, k0, k1):
            """rhs AP for matmul segment [k0:k1) of this subgroup's state."""
            if not isinstance(xs, tuple):
                return xs[:, k0:k1]
            x = sub["x"]
            if k1 <= x:
                return xs[0][:, k0:k1]
            assert k0 >= x, (k0, k1, x)
            return xs[1][:, k0 - x:k1 - x]

        def seg_bounds(sub):
            w = sub["w"]
            cuts = {0, w}
            cuts.update(range(512, w, 512))
            if sub["kind"] in ("m", "q") and 0 < sub["x"] < w:
                cuts.add(sub["x"])
            cs = sorted(cuts)
            return list(zip(cs[:-1], cs[1:]))

        X = []
        for gi, sub in enumerate(geo):
            xs = new_state(gi, sub)
            for part in (xs if isinstance(xs, tuple) else (xs,)):
                # d-inits on DVE (their pipelines start first; Pool's serial
                # memset queue would gate the first matmuls), a-inits on Pool
                if sub["kind"] == "d" and MEMSET_SPLIT:
                    nc.vector.memset(part, 1.0)
                else:
                    nc.gpsimd.memset(part, 1.0)
            X.append(xs)
        if W == 0:
            # no warmup: exact chunk-0 init goes straight into the initial state
            x0 = X[0][0] if isinstance(X[0], tuple) else X[0]
            nc.gpsimd.tensor_copy(out=x0[:, 0:BPC], in_=e0_sb)

        e16_t = e8_t = None
        bi = -1
        for g in range(G):
            hit = [b for b, (g0, s) in enumerate(batches) if g0 == g]
            if hit:
                bi = hit[0]
                g0, bs = batches[bi]
                if es16_d is not None and ES16_FIRST:
                    e16_t = e16pool.tile([S, KD * A16], mybir.dt.bfloat16)
                    nc.sync.dma_start(out=e16_t[:, 0:bs * A16],
                                      in_=es16_d[:, g0 * A16:(g0 + bs) * A16])
                if es8_d is not None:
                    e8_t = e8pool.tile([S, KD * A8], mybir.dt.float8e4)
                    nc.sync.dma_start(out=e8_t[:, 0:bs * A8],
                                      in_=es8_d[:, g0 * A8:(g0 + bs) * A8])
                if es16_d is not None and not ES16_FIRST:
                    e16_t = e16pool.tile([S, KD * A16], mybir.dt.bfloat16)
                    if bi < SPLIT_HEAD and bs == 1:
                        # per-group transfers for the leading slices: each
                        # pipeline starts on its own (much smaller) arrival
                        for sub in geo:
                            if sub["kind"] == "a":
                                eo, wsub = sub["eoff"], sub["w"]
                            elif sub["kind"] == "q" and sub["x"] > 0:
                                eo, wsub = sub["eoff"], sub["x"]
                            elif sub["kind"] == "m" and sub["x"] < sub["w"]:
                                eo, wsub = sub["eoff16"], sub["w"] - sub["x"]
                            else:
                                continue
                            nc.sync.dma_start(
                                out=e16_t[:, eo:eo + wsub],
                                in_=es16_d[:, g0 * A16 + eo:g0 * A16 + eo + wsub])
                    else:
                        nc.sync.dma_start(out=e16_t[:, 0:bs * A16],
                                          in_=es16_d[:, g0 * A16:(g0 + bs) * A16])
            j = g - batches[bi][0]

            final = (W > 0 and g == W - 1 and WOUT_BIG) or (g == G - 1 and YOUT_BIG)
            if final:
                # single wide tile so wout/yout go out as one DMA each
                Xbig = xpool.tile([S, N], mybir.dt.bfloat16, tag=f"xfin{g}")
            if SHARED_PS == "a":
                a_tot = sum(s["w"] for s in geo if s["kind"] == "a")
                psbig = pshared.tile([S, a_tot], mybir.dt.float32, tag="psa")
            elif SHARED_PS:
                psbig = pshared.tile([S, N], mybir.dt.float32, tag="ps")
            aoff = 0
            for gi, sub in enumerate(geo):
                w, col, eo, kind = sub["w"], sub["col"], sub["eoff"], sub["kind"]
                if SHARED_PS == "a" and kind == "a":
                    ps = psbig[:, aoff:aoff + w]
                    aoff += w
                elif SHARED_PS is True:
                    ps = psbig[:, col:col + w]
                else:
                    pst = ppools[gi].tile([S, w], mybir.dt.float32, tag=f"ps{gi}")
                    ps = pst[:]
                for k0, k1 in seg_bounds(sub):
                    nc.tensor.matmul(out=ps[:, k0:k1], lhsT=E_sb,
                                     rhs=state_seg(X[gi], sub, k0, k1),
                                     start=True, stop=True)
                if final and kind not in ("m", "q"):
                    Xn = Xbig[:, col:col + w]
                else:
                    Xn = new_state(gi, sub, g)
                if kind == "d":
                    es = e8_t[:, j * A8 + eo: j * A8 + eo + w]
                    nc.vector.tensor_mul(out=Xn, in0=ps, in1=es)
                elif kind == "a":
                    ev = evpool.tile([S, w], mybir.dt.bfloat16, tag=f"ev{gi}")
                    nc.scalar.copy(out=ev[:], in_=ps)
                    es = e16_t[:, j * A16 + eo: j * A16 + eo + w]
                    nc.vector.tensor_mul(out=Xn, in0=ev[:], in1=es)
                elif kind == "m":  # d-style [0:x] and a-style [x:w] parts
                    x, eo16 = sub["x"], sub["eoff16"]
                    es = e8_t[:, j * A8 + eo: j * A8 + eo + x]
                    nc.vector.tensor_mul(out=Xn[0], in0=ps[:, 0:x], in1=es)
                    ev = evpool.tile([S, w - x], mybir.dt.bfloat16, tag=f"ev{gi}")
                    nc.scalar.copy(out=ev[:], in_=ps[:, x:w])
                    es = e16_t[:, j * A16 + eo16: j * A16 + eo16 + (w - x)]
                    nc.vector.tensor_mul(out=Xn[1], in0=ev[:], in1=es)
                else:  # "q": one evac; DVE 2x on [0:x], Pool on [x:w]
                    x, eo8 = sub["x"], sub["eoff8"]
                    ev = evpool.tile([S, w], mybir.dt.bfloat16, tag=f"ev{gi}")
                    nc.scalar.copy(out=ev[:], in_=ps)
                    if x > 0:
                        es = e16_t[:, j * A16 + eo: j * A16 + eo + x]
                        nc.vector.tensor_mul(out=Xn[0] if isinstance(Xn, tuple)
                                             else Xn, in0=ev[:, 0:x], in1=es)
                    if x < w:
                        es = e8_t[:, j * A8 + eo8: j * A8 + eo8 + (w - x)]
                        nc.gpsimd.tensor_mul(out=Xn[1] if isinstance(Xn, tuple)
                                             else Xn, in0=ev[:, x:w], in1=es)
                X[gi] = Xn
            if W > 0 and g == W - 1:
                # exact chunk-0 init via Pool copy (keeps SP/HWDGE free); the
                # wout snapshot skips cols 0:BPC (unused in the stitch) so the
                # two touch disjoint regions and don't serialize
                nc.gpsimd.tensor_copy(out=X[0][:, 0:BPC], in_=e0_sb)
                wsnap = list(X)
            if W > 0 and g == W + 1:
                # wout issued 2 slices late so its sem wait never starves the
                # SP queue of upcoming es batches
                for gi, sub in enumerate(geo):
                    lo = BPC if gi == 0 else 0
                    nc.sync.dma_start(
                        out=wout_d[:, sub["col"] + lo:sub["col"] + sub["w"]],
                        in_=wsnap[gi][:, lo:sub["w"]])
        if YOUT_BIG and not any(s["kind"] == "m" for s in geo):
            nc.sync.dma_start(out=yout_d[:], in_=Xbig[:])
        else:
            for gi, sub in enumerate(geo):
                c0 = sub["col"]
                if isinstance(X[gi], tuple):
                    x = sub["x"]
                    nc.sync.dma_start(out=yout_d[:, c0:c0 + x], in_=X[gi][0])
                    nc.sync.dma_start(out=yout_d[:, c0 + x:c0 + sub["w"]],
                                      in_=X[gi][1])
                else:
                    nc.sync.dma_start(
                        out=yout_d[:, c0:c0 + sub["w"]], in_=X[gi])
    nc.compile()
    return nc


LAST_RESULTS = None
LAST_NC = None


def _chunk_meta():
    geo, _, _ = _group_geometry()
    beta = np.empty(C)
    for c in range(C):
        tag, _ = _chunk_slot(geo, c)
        beta[c] = BETA16 if tag == "16" else BETA8
    return beta


def prep_in_maps(log_observation: np.ndarray, log_transition: np.ndarray):
    geo, A16, A8 = _group_geometry()
    E = np.exp(log_transition.astype(np.float64)).astype(np.float32)
    ew_bf = E.astype(bf16)

    e8_full = np.exp(log_observation.astype(np.float32) - BETA8)  # [B,T,S] f32
    sc = np.float32(np.exp(BETA8 - BETA16))

    in_maps = []
    for k in range(NCORES):
        blk8 = e8_full[k * BPC:(k + 1) * BPC]          # [BPC, T, S] f32
        if W > 0:
            pad8 = np.ones((S, BPC, T + W), dtype=np.float32)
            pad8[:, :, W - 1:W - 1 + T] = blk8.transpose(2, 0, 1)
        else:
            # t = L*c + g + 1: slice stream starts at obs[:,1]; obs[:,0] is
            # consumed by the exact e0 init
            pad8 = np.ones((S, BPC, T), dtype=np.float32)
            pad8[:, :, 0:T - 1] = blk8[:, 1:, :].transpose(2, 0, 1)
        st_s, st_b, st_t = pad8.strides
        view = np.lib.stride_tricks.as_strided(
            pad8, shape=(G, S, C, BPC), strides=(st_t, st_s, L * st_t, st_b)
        )  # [g, s, c, b] fp32

        es16 = np.ones((G, S, A16), dtype=bf16) if A16 else None
        es8 = np.ones((G, S, A8), dtype=fp8) if A8 else None
        for sub in geo:
            c0 = sub["col"] // BPC
            parts = [(sub["kind"], 0, sub["w"], sub["eoff"])]
            if sub["kind"] == "m":
                x = sub["x"]
                parts = [("d", 0, x, sub["eoff"]),
                         ("a", x, sub["w"] - x, sub["eoff16"])]
            elif sub["kind"] == "q":
                x = sub["x"]
                parts = [("a", 0, x, sub["eoff"]),
                         ("d", x, sub["w"] - x, sub["eoff8"])]
                parts = [p for p in parts if p[2] > 0]
            for knd, off, pw, eo in parts:
                v = view[:, :, c0 + off // BPC:c0 + (off + pw) // BPC, :]
                v = v.reshape(G, S, pw)
                if knd == "a":
                    es16[:, :, eo:eo + pw] = (v * sc).astype(bf16)
                else:
                    es8[:, :, eo:eo + pw] = np.clip(
                        v, CLIP_LO, CLIP_HI).astype(fp8)
        # the final chunk's g=G-1 step is a pad: force its factor to exactly 1
        tag, lo = _chunk_slot(geo, C - 1)
        if tag == "16":
            es16[G - 1, :, lo:lo + BPC] = 1.0
        else:
            es8[G - 1, :, lo:lo + BPC] = 1.0
        m = {}
        if A16:
            m["es16"] = np.ascontiguousarray(
                es16.transpose(1, 0, 2)).reshape(S, G * A16)
        if A8:
            m["es8"] = np.ascontiguousarray(
                es8.transpose(1, 0, 2)).reshape(S, G * A8)
        beta0 = BETA16 if _chunk_slot(geo, 0)[0] == "16" else BETA8
        e0 = np.exp(log_observation[k * BPC:(k + 1) * BPC, 0, :].astype(np.float32).T
                    - beta0)
        m["cst"] = np.concatenate([ew_bf, e0.astype(bf16)], axis=1)
        in_maps.append(m)
    return in_maps


def stitch_outputs(results) -> np.ndarray:
    betc = _chunk_meta()                    # [C]
    cnt = np.full(C, G, dtype=np.float64)
    cnt[0] = L + 1
    cnt[C - 1] = G - 1
    out = np.empty(B, dtype=np.float64)
    for k in range(NCORES):
        y = results[k]["yout"].astype(np.float64).reshape(S, C, BPC)
        Sy = y.sum(axis=0)            # [C, BPC]
        if W == 0:
            Sw = np.full((C, BPC), float(S))
        else:
            w = results[k]["wout"].astype(np.float64).reshape(S, C, BPC)
            Sw = np.maximum(w.sum(axis=0), 1e-300)  # chunk-0 cols unused
        ly = np.log(Sy) + (betc * cnt)[:, None]
        lw = np.log(Sw) + (betc * W)[:, None]
        logZ = ly[C - 1] + np.sum(ly[: C - 1] - lw[1:], axis=0)
        out[k * BPC:(k + 1) * BPC] = -logZ
    return out


def kernel(log_observation: np.ndarray, log_transition: np.ndarray) -> np.ndarray:
    assert log_observation.shape == (B, T, S)
    assert log_transition.shape == (S, S)

    in_maps = prep_in_maps(log_observation, log_transition)
    nc = _build_device_program()
    res = run_bass_kernel_spmd(nc, in_maps, core_ids=list(range(NCORES)))
    global LAST_RESULTS, LAST_NC
    LAST_RESULTS = res
    LAST_NC = nc
    return stitch_outputs(res.results).astype(np.float32)


if __name__ == "__main__":
    rng = np.random.default_rng(0)
    obs = rng.standard_normal((B, T, S), dtype=np.float32)
    lt = rng.standard_normal((S, S), dtype=np.float32)
    lt = lt - np.log(np.exp(lt).sum(axis=1, keepdims=True))
    print(kernel(obs, lt)[:4])
